# revision 3
# baseline (speedup 1.0000x reference)
"""LSTM layer kernel for Trainium2 (8 NeuronCores, Bass/Tile).

Reference computation (fp32):
    z = concat([x, h], axis=1)                 # [B, IN+OUT]
    f = sigmoid(z @ w_f + b_f)
    i = sigmoid(z @ w_i + b_i)
    g = tanh   (z @ w_c + b_c)
    o = sigmoid(z @ w_o + b_o)
    c_new = c * f + i * g
    h_new = tanh(c_new) * o                    # [B, OUT]

Shapes: B=4096, IN=OUT=1024, K=IN+OUT=2048.

Sharding (8 cores): 2-D grid, 4 batch-groups x 2 output-column-groups.
Core (i, j) computes h_new[i*1024:(i+1)*1024, j*512:(j+1)*512].
Per-core HBM traffic (bf16 matmul operands): 4 MiB zT + 8 MiB weights
+ 2 MiB cT + 2 MiB out = 16 MiB.  No collectives.

Device layout: contraction dim (k) and the output-channel dim (o) sit on
SBUF partitions:
    zT [2048, 1024] bf16 (k, b)     - moving operand
    w  [4, 2048, 4, 128] bf16 (oc, k, gate, p) - stationary operand,
         oc-major so any ko-slice of one oc is HBM-contiguous
    out = w.T @ zT -> psum [o, b]; per-o bias is a per-partition ACT bias
    and sigmoid/tanh run straight out of PSUM.
The host pre-transposes/casts the shards and transposes the per-core
h_newT shards back when assembling the full output.

bf16 matmuls: 1 cycle/row at N=512 (same as f32r) but the stationary
operand qualifies for fast weight load + the PE's ldweights hoisting,
unlike f32r whose matmul must self-load weights serially.  Quantizing
z and w to bf16 costs ~1.9e-3 rel err (measured vs fp32 reference),
well inside the 2e-2 gate; c/bias/psum/activations stay fp32.

Loop structure: per oc, two batch-half passes (nb), each ko-outer x
gate-inner so one z chunk feeds 4 matmuls into 4 psum banks.  The two
passes alternate bank groups so ACT drains one group while the PE fills
the other; z chunks are consumed in DMA arrival order, which keeps the
first oc pass PE-paced ~2us in.
"""

import numpy as np

import concourse.bass as bass
import concourse.tile as tile
from concourse import bacc
from concourse import mybir
from concourse.bass_utils import run_bass_kernel_spmd

P = 128
B_FULL, IN, OUT = 4096, 1024, 1024
K = IN + OUT                 # 2048 contraction
RB, RO = 4, 2                # batch-shards x out-col-shards = 8 cores
B_L = B_FULL // RB           # 1024 batch rows per core
O_L = OUT // RO              # 512 out cols per core
KO = K // P                  # 16 k-chunks
OC = O_L // P                # 4 out chunks per core
NG = 4                       # gates
NT = 512                     # moving free dim per matmul
NB = B_L // NT               # 2 batch tiles

F32 = mybir.dt.float32
BF16 = mybir.dt.bfloat16
GATES = ("f", "i", "c", "o")

# exec time of the most recent traced run (ns); set by _run when trace=True
last_exec_time_ns = None

_NC_CACHE = {}


def _build_nc(loop_r=None):
    # loop_r: timing-only variant that repeats the whole body in a hardware
    # For_i loop (see test.py hw_loop_slope).
    nc = bacc.Bacc()

    zT = nc.dram_tensor("zT", [K, B_L], BF16, kind="ExternalInput")
    cT = nc.dram_tensor("cT", [O_L, B_L], F32, kind="ExternalInput")
    # oc-major gate-fused weights: [oc, k, gate, p], o_local = oc*128 + p
    wA = nc.dram_tensor("wA", [OC, K, NG, P], BF16, kind="ExternalInput")
    # gate-fused biases: [p, oc, gate]
    bA = nc.dram_tensor("bA", [P, OC, NG], F32, kind="ExternalInput")
    hT = nc.dram_tensor("hT", [O_L, B_L], F32, kind="ExternalOutput")

    zT_t = zT[:, :].rearrange("(ko kp) b -> kp ko b", kp=P)    # [128,16,1024]
    cT_t = cT[:, :].rearrange("(oc p) b -> p oc b", p=P)       # [128,4,1024]
    hT_t = hT[:, :].rearrange("(oc p) b -> p oc b", p=P)
    wA_t = wA[:, :, :, :].rearrange(
        "oc (ko kp) g p -> kp oc ko (g p)", kp=P
    )                                                          # [128,4,16,512]

    sig = mybir.ActivationFunctionType.Sigmoid
    tanh = mybir.ActivationFunctionType.Tanh

    import contextlib

    with tile.TileContext(nc) as tc:
        with (
            tc.For_i(0, loop_r, 1) if loop_r else contextlib.nullcontext(),
            tc.tile_pool(name="zpool", bufs=2) as zpool,
            tc.tile_pool(name="cpool", bufs=2) as cpool,
            tc.tile_pool(name="bpool", bufs=1) as bpool,
            tc.tile_pool(name="wpool", bufs=3) as wpool,
            tc.tile_pool(name="gates", bufs=2) as gpool,
            tc.tile_pool(name="temps", bufs=2) as tpool,
            tc.tile_pool(name="psum", bufs=1, space="PSUM") as psum_pool,
        ):
            z_sb = zpool.tile([P, KO, B_L], BF16, tag="z")     # 4 MiB
            w_tiles = [
                wpool.tile([P, KO, NG * P], BF16, tag="w", name=f"w_oc{oc}")
                for oc in range(OC)
            ]
            c_tiles = [
                cpool.tile([P, B_L], F32, tag="c", name=f"c_oc{oc}")
                for oc in range(OC)
            ]
            b_sb = bpool.tile([P, OC, NG], F32)

            # DMA schedule.  Sync-ring FIFO order == arrival order:
            # z and w0 stream together in 2-ko units (each 256 KiB, HBM
            # contiguous) so the first oc/nb pass can start ~2us in and
            # stays ahead of the PE; later weight/c tiles follow as whole
            # contiguous blocks, all well before the PE needs them.
            # Bias + c0 ride the scalar ring so they don't delay z.
            nc.scalar.dma_start(b_sb[:, :, :], bA[:, :, :])
            nc.scalar.dma_start(c_tiles[0][:, :], cT_t[:, 0, :])
            KU = 2                                             # ko per unit
            for ku in range(KO // KU):
                ks = slice(ku * KU, (ku + 1) * KU)
                nc.sync.dma_start(z_sb[:, ks, :], zT_t[:, ks, :])
                nc.sync.dma_start(w_tiles[0][:, ks, :], wA_t[:, 0, ks, :])
            nc.sync.dma_start(w_tiles[1][:, :, :], wA_t[:, 1, :, :])
            nc.sync.dma_start(c_tiles[1][:, :], cT_t[:, 1, :])
            nc.sync.dma_start(w_tiles[2][:, :, :], wA_t[:, 2, :, :])
            nc.sync.dma_start(c_tiles[2][:, :], cT_t[:, 2, :])
            nc.sync.dma_start(w_tiles[3][:, :, :], wA_t[:, 3, :, :])
            nc.sync.dma_start(c_tiles[3][:, :], cT_t[:, 3, :])

            for oc in range(OC):
                w_sb = w_tiles[oc]
                c_sb = c_tiles[oc]

                for nb in range(NB):
                    bsl = slice(nb * NT, (nb + 1) * NT)
                    ps = {
                        g: psum_pool.tile([P, NT], F32, tag=f"ps_{g}{nb}",
                                          name=f"ps_{g}{nb}")
                        for g in GATES
                    }
                    for ko in range(KO):
                        for gi, g in enumerate(GATES):
                            nc.tensor.matmul(
                                ps[g][:, :],
                                lhsT=w_sb[:, ko, gi * P:(gi + 1) * P],
                                rhs=z_sb[:, ko, bsl],
                                start=(ko == 0),
                                stop=(ko == KO - 1),
                            )
                    gate_sb = {}
                    for gi, g in enumerate(GATES):
                        gt = gpool.tile(
                            [P, NT], F32, tag=f"gate_{g}{nb}",
                            name=f"gate_{g}{nb}",
                        )
                        nc.scalar.activation(
                            gt[:, :], ps[g][:, :],
                            tanh if g == "c" else sig,
                            bias=b_sb[:, oc, gi:gi + 1],
                        )
                        gate_sb[g] = gt

                    cf = tpool.tile([P, NT], F32, tag=f"cf{nb}",
                                    name=f"cf{nb}")
                    ig = tpool.tile([P, NT], F32, tag=f"ig{nb}",
                                    name=f"ig{nb}")
                    nc.vector.tensor_mul(
                        cf[:, :], c_sb[:, bsl], gate_sb["f"][:, :]
                    )
                    nc.vector.tensor_mul(
                        ig[:, :], gate_sb["i"][:, :], gate_sb["c"][:, :]
                    )
                    nc.vector.tensor_add(cf[:, :], cf[:, :], ig[:, :])
                    nc.scalar.activation(cf[:, :], cf[:, :], tanh)
                    nc.vector.tensor_mul(
                        cf[:, :], cf[:, :], gate_sb["o"][:, :]
                    )
                    nc.scalar.dma_start(hT_t[:, oc, bsl], cf[:, :])

    # run the Bacc pass pipeline (alloc_regs, wait-splitting, ...);
    # run_bass_via_pjrt does not finalize on our behalf
    nc.finalize()
    return nc


def _get_nc():
    if "nc" not in _NC_CACHE:
        _NC_CACHE["nc"] = _build_nc()
    return _NC_CACHE["nc"]


def _shard_inputs(x, h, c, w_f, b_f, w_i, b_i, w_c, b_c, w_o, b_o):
    import ml_dtypes

    ws = {"f": w_f, "i": w_i, "c": w_c, "o": w_o}
    bz = {"f": b_f, "i": b_i, "c": b_c, "o": b_o}
    f32 = np.float32
    bf16 = ml_dtypes.bfloat16

    # per-out-group fused weight/bias shards (shared by the 4 batch groups)
    # wA[oc, k, g, p] = w_g[k, j*O_L + oc*P + p]
    wA_sh = {}
    bA_sh = {}
    for j in range(RO):
        cols = slice(j * O_L, (j + 1) * O_L)
        # [K, OC, NG, P] -> transpose to [OC, K, NG, P]
        wj = np.stack(
            [np.asarray(ws[g][:, cols], dtype=f32).reshape(K, OC, P)
             for g in GATES],
            axis=2,
        )                                       # [K, OC, NG, P]
        wA_sh[j] = np.ascontiguousarray(
            wj.transpose(1, 0, 2, 3)
        ).astype(bf16)                          # [OC, K, NG, P]
        bA_sh[j] = np.ascontiguousarray(
            np.stack(
                [np.asarray(bz[g], dtype=f32).reshape(-1)[cols].reshape(OC, P).T
                 for g in GATES],
                axis=2,
            )
        )
    in_maps = []
    for i in range(RB):
        rows = slice(i * B_L, (i + 1) * B_L)
        zT = np.ascontiguousarray(
            np.concatenate([x[rows], h[rows]], axis=1).T.astype(bf16)
        )
        for j in range(RO):
            cT = np.ascontiguousarray(
                c[rows, j * O_L:(j + 1) * O_L].T, dtype=f32
            )
            in_maps.append(
                {"zT": zT, "cT": cT, "wA": wA_sh[j], "bA": bA_sh[j]}
            )
    return in_maps


def _run(in_maps, trace=False, trace_cores=None):
    global last_exec_time_ns
    nc = _get_nc()
    res = run_bass_kernel_spmd(
        nc, in_maps, list(range(RB * RO)),
        trace=trace, trace_cores=trace_cores,
    )
    if trace:
        last_exec_time_ns = res.exec_time_ns
    return res.results


def kernel(x, h, c, w_f, b_f, w_i, b_i, w_c, b_c, w_o, b_o):
    in_maps = _shard_inputs(
        x, h, c, w_f, b_f, w_i, b_i, w_c, b_c, w_o, b_o
    )
    results = _run(in_maps)
    out = np.empty((B_FULL, OUT), np.float32)
    for i in range(RB):
        for j in range(RO):
            shard = results[i * RO + j]["hT"]  # [O_L, B_L]
            out[i * B_L:(i + 1) * B_L, j * O_L:(j + 1) * O_L] = shard.T
    return out
